# revision 24
# baseline (speedup 1.0000x reference)
"""GATv2Conv + edgeMLP (NodeEdgeLayer) Bass kernel for 8 TRN2 NeuronCores.

Strategy (SPMD, one program, per-core data):
  - Nodes are dst-partitioned: core c owns 6272 consecutive node ids
    (N padded 50000 -> 50176 = 8*49*128). Each core owns 49 blocks of
    128 nodes.
  - Edges are routed to the core owning their dst, sorted by dst block,
    and padded so every (core, block) holds exactly C edge slots
    (C = global max block load, rounded up to 128). All segment softmax
    math is block-local: one-hot matrices P/PT map the 128 edges of a
    tile to the 128 nodes of its block on the tensor engine.
  - Softmax denominators are divided at the node level (agg_raw/denom)
    instead of per-edge alpha, which is algebraically identical to the
    reference (including the +1e-16 term).
  - Stage 2 (edge MLP) needs x_new for arbitrary src: per-core x_new
    shards are AllGathered in 7 chunks (overlapped with stage-1 compute)
    into a gather table laid out in a permuted node order pi() chosen so
    chunked AllGather outputs land contiguously.  All node indices the
    device ever sees are pre-permuted on the host.
"""

import sys

sys.path.insert(0, "/opt/trn_rl_repo")

import numpy as np

import concourse.bass as bass
import concourse.bacc as bacc
import concourse.mybir as mybir
import concourse.tile as tile
from concourse.bass_utils import run_bass_kernel_spmd

F32 = mybir.dt.float32
I32 = mybir.dt.int32

from contextlib import nullcontext as _nullctx

NCORES = 8
N, E, D, H = 50000, 500000, 128, 4
HD = D // H
LN_EPS = 1e-5
NB = 49            # node blocks per core
NL = NB * 128      # nodes per core (6272)
NPAD = NCORES * NL # padded node count (50176)
CHUNKS = 7         # allgather chunks
BPC = NB // CHUNKS # blocks per chunk (7)


# ----------------------------------------------------------------- host prep

def _pi_rows():
    """Permuted table row for node (r, b, j):  g*8*BPC*128 + r*BPC*128 + k*128 + j
    where g = b // BPC, k = b % BPC.  Returns pi as array over original ids."""
    n = np.arange(NPAD)
    r = n // NL
    local = n % NL
    b = local // 128
    j = local % 128
    g = b // BPC
    k = b % BPC
    return ((g * NCORES + r) * BPC + k) * 128 + j


def _prep(x, edge_index, edge_attr):
    src = np.asarray(edge_index[0])
    dst = np.asarray(edge_index[1])
    ea = np.asarray(edge_attr)
    x = np.asarray(x)

    pi = _pi_rows()                       # original node id -> table row
    x_pi = np.zeros((NPAD, D), np.float32)
    x_pi[pi[:N]] = x                      # permuted gather table for stage 1

    core_of = dst // NL                   # owner core per edge
    blk_of = (dst % NL) // 128            # block within core

    # per (core, block) edge lists
    order = np.argsort(core_of * NB + blk_of, kind="stable")
    counts = np.bincount(core_of * NB + blk_of, minlength=NCORES * NB)
    C = int(np.ceil(counts.max() / 128) * 128)
    EP = NB * C                           # padded edges per core
    starts = np.zeros(NCORES * NB + 1, np.int64)
    np.cumsum(counts, out=starts[1:])

    src_pad = np.zeros((NCORES, EP), np.int32)
    dstloc_pad = np.zeros((NCORES, EP), np.float32)
    pen_pad = np.full((NCORES, EP), -1e30, np.float32)
    ea_pad = np.zeros((NCORES, EP, D), np.float32)
    pos_of_edge = np.empty(E, np.int64)   # (core, padded position) flattened

    src_pi = pi[src]
    for c in range(NCORES):
        for b in range(NB):
            i0, i1 = starts[c * NB + b], starts[c * NB + b + 1]
            idx = order[i0:i1]
            k = i1 - i0
            p0 = b * C
            src_pad[c, p0:p0 + k] = src_pi[idx]
            dstloc_pad[c, p0:p0 + k] = (dst[idx] % 128).astype(np.float32)
            pen_pad[c, p0:p0 + k] = 0.0
            ea_pad[c, p0:p0 + k] = ea[idx]
            pos_of_edge[idx] = c * EP + p0 + np.arange(k)

    # device layouts
    # gather/index arrays: [128, T] with column t = slots [t*128, (t+1)*128)
    T = EP // 128
    src_dev = src_pad.reshape(NCORES, T, 128).transpose(0, 2, 1).copy()
    dstloc_dev = dstloc_pad.reshape(NCORES, T, 128).transpose(0, 2, 1).copy()
    pen_dev = pen_pad.reshape(NCORES, T, 128).transpose(0, 2, 1).copy()
    eaT_dev = ea_pad.transpose(0, 2, 1).copy()            # [NC, 128, EP]

    # xT_local: [NC, 128, NL], column b*128+j = features of node (c, b, j)
    xp = np.zeros((NPAD, D), np.float32)
    xp[:N] = x
    xT_loc = xp.reshape(NCORES, NL, D).transpose(0, 2, 1).copy()

    # stage-1 endpoint features pre-distributed per edge shard, transposed
    xsT_dev = np.stack([x_pi[src_pad[c]].T for c in range(NCORES)])  # [NC,128,EP]

    return dict(x_pi=x_pi, src_dev=src_dev, dstloc_dev=dstloc_dev,
                pen_dev=pen_dev, eaT_dev=eaT_dev, xT_loc=xT_loc, xsT_dev=xsT_dev,
                C=C, EP=EP, T=T, pos_of_edge=pos_of_edge)


# ------------------------------------------------------------- device program

def build_program(C, has_bsum, has_bias, has_bmlp, fold_ln,
                  n_blocks=NB, with_collective=True, trn="TRN2", reps=1):
    """One SPMD program. C = edge slots per block.  reps>1 repeats each
    stage's body in a hardware For_i loop (benchmarking only)."""
    TB = C // 128                  # tiles per block
    T = n_blocks * TB              # tiles per core
    EP = n_blocks * C

    nc = bacc.Bacc(trn, target_bir_lowering=False, debug=False,
                   num_devices=NCORES)

    # ---- dram I/O
    x_pi = nc.dram_tensor("x_pi", [NPAD, D], F32, kind="ExternalInput")
    xsT_all = nc.dram_tensor("xsT_all", [D, EP], F32, kind="ExternalInput")
    xT_loc = nc.dram_tensor("xT_loc", [D, NL], F32, kind="ExternalInput")
    eaT = nc.dram_tensor("eaT", [D, EP], F32, kind="ExternalInput")
    src_i = nc.dram_tensor("src_i", [128, T], I32, kind="ExternalInput")
    dstloc = nc.dram_tensor("dstloc", [128, T], F32, kind="ExternalInput")
    pen = nc.dram_tensor("pen", [128, T], F32, kind="ExternalInput")
    w_l = nc.dram_tensor("w_l", [D, D], F32, kind="ExternalInput")
    w_r = nc.dram_tensor("w_r", [D, D], F32, kind="ExternalInput")
    w_e = nc.dram_tensor("w_e", [D, D], F32, kind="ExternalInput")
    w_mlp = nc.dram_tensor("w_mlp", [2 * D, D], F32, kind="ExternalInput")
    att_b = nc.dram_tensor("att_b", [128, D], F32, kind="ExternalInput")
    iota_f = nc.dram_tensor("iota_f", [128, 128], F32, kind="ExternalInput")
    ident = nc.dram_tensor("ident", [128, 128], F32, kind="ExternalInput")
    rows1 = nc.dram_tensor("rows1", [1, 128], F32, kind="ExternalInput")  # ones
    bsum_r = nc.dram_tensor("bsum_r", [1, 128], F32, kind="ExternalInput")
    bias_b = nc.dram_tensor("bias_b", [128, 128], F32, kind="ExternalInput")
    bmlp_r = nc.dram_tensor("bmlp_r", [1, 128], F32, kind="ExternalInput")
    lng_b = nc.dram_tensor("lng_b", [128, 2 * D], F32, kind="ExternalInput")
    lnb_b = nc.dram_tensor("lnb_b", [128, 2 * D], F32, kind="ExternalInput")

    xnew_out = nc.dram_tensor("xnew_out", [NL, D], F32, kind="ExternalOutput")
    eaT_out = nc.dram_tensor("eaT_out", [D, EP], F32, kind="ExternalOutput")

    with tile.TileContext(nc) as tc:
        with (
            tc.tile_pool(name="const", bufs=1) as cp,
            tc.tile_pool(name="dram", bufs=1, space="DRAM") as dr,
        ):
            # ---- resident constants
            c_w_l = cp.tile([D, D], F32); nc.sync.dma_start(out=c_w_l[:], in_=w_l[:, :])
            c_w_r = cp.tile([D, D], F32); nc.sync.dma_start(out=c_w_r[:], in_=w_r[:, :])
            c_w_e = cp.tile([D, D], F32); nc.sync.dma_start(out=c_w_e[:], in_=w_e[:, :])
            c_wm = cp.tile([D, 2 * D], F32)  # [k-chunk, 2 chunks of 128 cols]
            nc.sync.dma_start(out=c_wm[:, 0:128], in_=w_mlp[0:128, :])
            nc.sync.dma_start(out=c_wm[:, 128:256], in_=w_mlp[128:256, :])
            c_att = cp.tile([128, D], F32); nc.sync.dma_start(out=c_att[:], in_=att_b[:, :])
            c_iota = cp.tile([128, 128], F32); nc.sync.dma_start(out=c_iota[:], in_=iota_f[:, :])
            c_id = cp.tile([128, 128], F32); nc.sync.dma_start(out=c_id[:], in_=ident[:, :])
            c_ones = cp.tile([1, 128], F32); nc.sync.dma_start(out=c_ones[:], in_=rows1[:, :])
            c_bsum = cp.tile([1, 128], F32); nc.sync.dma_start(out=c_bsum[:], in_=bsum_r[:, :])
            c_bias = cp.tile([128, 128], F32); nc.sync.dma_start(out=c_bias[:], in_=bias_b[:, :])
            c_bmlp = cp.tile([1, 128], F32); nc.sync.dma_start(out=c_bmlp[:], in_=bmlp_r[:, :])
            if not fold_ln:
                c_lng = cp.tile([128, 2 * D], F32); nc.sync.dma_start(out=c_lng[:], in_=lng_b[:, :])
                c_lnb = cp.tile([128, 2 * D], F32); nc.sync.dma_start(out=c_lnb[:], in_=lnb_b[:, :])
            c_eps = cp.tile([128, 1], F32); nc.vector.memset(c_eps[:], LN_EPS)
            c_src = cp.tile([128, T], I32); nc.sync.dma_start(out=c_src[:], in_=src_i[:, :])
            c_dstl = cp.tile([128, T], F32); nc.sync.dma_start(out=c_dstl[:], in_=dstloc[:, :])
            c_pen = cp.tile([128, T], F32); nc.sync.dma_start(out=c_pen[:], in_=pen[:, :])
            c_xT = cp.tile([D, NL], F32); nc.sync.dma_start(out=c_xT[:], in_=xT_loc[:, :])
            xnew_sb = cp.tile([128, NL], F32)   # [j, b*128+f]

            ag_in = dr.tile([NL, D], F32)
            ag_out_t = nc.dram_tensor("ag_out", [NPAD, D], F32, addr_space="Shared")
            ag_out = ag_out_t.ap()

            # =========================== stage 1 ===========================
            with (
                tc.tile_pool(name="s1", bufs=4) as sp,
                tc.tile_pool(name="s1ea", bufs=2) as ep,
                tc.tile_pool(name="s1g", bufs=6) as gp,
                tc.tile_pool(name="ps_t", bufs=4, space="PSUM") as ps_t,
                tc.tile_pool(name="ps_m", bufs=2, space="PSUM") as ps_m,
                tc.tile_pool(name="ps_a", bufs=2, space="PSUM") as ps_a,
                (tc.For_i(0, reps, 1) if reps > 1 else _nullctx()),
            ):
                for b in range(n_blocks):
                    ea_blk = ep.tile([128, C], F32, tag="ea")
                    nc.sync.dma_start(out=ea_blk[:], in_=eaT[:, b * C:(b + 1) * C])
                    xs_blk = ep.tile([128, C], F32, tag="xs")
                    nc.sync.dma_start(out=xs_blk[:], in_=xsT_all[:, b * C:(b + 1) * C])
                    # xr for this block's nodes: [j, f]
                    xr_ps = ps_t.tile([128, 128], F32, space="PSUM", tag="tp")
                    nc.tensor.matmul(out=xr_ps[:], lhsT=c_xT[:, b * 128:(b + 1) * 128],
                                     rhs=c_w_r[:], start=True, stop=True)
                    xr_blk = sp.tile([128, 128], F32, tag="xr")
                    nc.scalar.copy(out=xr_blk[:], in_=xr_ps[:])

                    agg_ps = ps_a.tile([128, 132], F32, space="PSUM", tag="agg")

                    for tt in range(TB):
                        t = b * TB + tt
                        # x[src]^T tile: host-pregathered, [k, e]
                        xsT = xs_blk[:, tt * 128:(tt + 1) * 128]
                        # one-hots: PT [e, j], P [j, e]
                        pt = sp.tile([128, 128], F32, tag="pt")
                        nc.vector.tensor_tensor(
                            out=pt[:], in0=c_dstl[:, t:t + 1].to_broadcast([128, 128]),
                            in1=c_iota[:], op=mybir.AluOpType.is_equal)
                        p_ps = ps_t.tile([128, 128], F32, space="PSUM", tag="tp")
                        nc.tensor.transpose(out=p_ps[:], in_=pt[:], identity=c_id[:])
                        pm = sp.tile([128, 128], F32, tag="pm")
                        nc.vector.tensor_copy(out=pm[:], in_=p_ps[:])

                        # one PSUM bank: m in [:, 0:128], xl standalone in [:, 128:256]
                        mx_ps = ps_m.tile([128, 256], F32, space="PSUM", tag="m")
                        m_ps = mx_ps[:, 0:128]
                        xl_ps = mx_ps[:, 128:256]
                        nc.tensor.matmul(out=m_ps, lhsT=ea_blk[:, tt * 128:(tt + 1) * 128],
                                         rhs=c_w_e[:], start=True, stop=False)
                        nc.tensor.matmul(out=m_ps, lhsT=pm[:], rhs=xr_blk[:],
                                         start=False, stop=False)
                        nc.tensor.matmul(out=m_ps, lhsT=xsT[:], rhs=c_w_l[:],
                                         start=False, stop=not has_bsum,
                                         skip_group_check=True)
                        if has_bsum:
                            nc.tensor.matmul(out=m_ps, lhsT=c_ones[:], rhs=c_bsum[:],
                                             start=False, stop=True, skip_group_check=True)
                        # xl standalone for aggregation values (same bank, start=False
                        # overwrites since its has_written bits were cleared by ee's start)
                        nc.tensor.matmul(out=xl_ps, lhsT=xsT[:], rhs=c_w_l[:],
                                         start=False, stop=True, skip_group_check=True)

                        # leaky_relu(m) * att, reduce heads
                        t1 = sp.tile([128, 128], F32, tag="t1")
                        nc.scalar.mul(out=t1[:], in_=m_ps[:], mul=0.2)
                        lr = sp.tile([128, 128], F32, tag="lr")
                        nc.vector.tensor_max(out=lr[:], in0=t1[:], in1=m_ps[:])
                        s = sp.tile([128, 128], F32, tag="s")
                        nc.vector.tensor_mul(out=s[:], in0=lr[:], in1=c_att[:])
                        esc = sp.tile([128, H], F32, tag="esc")
                        nc.vector.tensor_reduce(
                            out=esc[:], in_=s[:].rearrange("e (h d) -> e h d", h=H),
                            axis=mybir.AxisListType.X, op=mybir.AluOpType.add)
                        ex = sp.tile([128, H], F32, tag="ex")
                        nc.scalar.activation(out=ex[:], in_=esc[:],
                                             func=mybir.ActivationFunctionType.Exp,
                                             bias=c_pen[:, t:t + 1])
                        # v = ex (head-bcast) * xl
                        v = sp.tile([128, 128], F32, tag="v")
                        nc.vector.tensor_tensor(
                            out=v[:].rearrange("e (h d) -> e h d", h=H),
                            in0=xl_ps[:].rearrange("e (h d) -> e h d", h=H),
                            in1=ex[:].rearrange("e (h o) -> e h o", o=1).to_broadcast([128, H, HD]),
                            op=mybir.AluOpType.mult)
                        # accumulate denom [j, 0:4] then agg [j, 4:132]
                        nc.tensor.matmul(out=agg_ps[:, 0:H], lhsT=pt[:], rhs=ex[:],
                                         start=(tt == 0), stop=(tt == TB - 1))
                        nc.tensor.matmul(out=agg_ps[:, H:H + 128], lhsT=pt[:], rhs=v[:],
                                         start=False, stop=(tt == TB - 1))

                    # block epilogue: x_new = agg/denom (+bias)
                    rec = sp.tile([128, H], F32, tag="rec")
                    nc.vector.tensor_scalar_add(out=rec[:], in0=agg_ps[:, 0:H], scalar1=1e-16)
                    nc.vector.reciprocal(out=rec[:], in_=rec[:])
                    xnb = xnew_sb[:, b * 128:(b + 1) * 128]
                    nc.vector.tensor_tensor(
                        out=xnb.rearrange("j (h d) -> j h d", h=H),
                        in0=agg_ps[:, H:H + 128].rearrange("j (h d) -> j h d", h=H),
                        in1=rec[:].rearrange("j (h o) -> j h o", o=1).to_broadcast([128, H, HD]),
                        op=mybir.AluOpType.mult)
                    if has_bias:
                        nc.vector.tensor_add(out=xnb, in0=xnb, in1=c_bias[:])

                    # chunk boundary: ship blocks [g*BPC, (g+1)*BPC) and allgather
                    if with_collective and (b % BPC == BPC - 1):
                        g = b // BPC
                        nc.sync.dma_start(
                            out=ag_in[g * BPC * 128:(g + 1) * BPC * 128, :]
                                .rearrange("(b j) f -> j b f", j=128),
                            in_=xnew_sb[:, g * BPC * 128:(g + 1) * BPC * 128]
                                .rearrange("j (b f) -> j b f", f=128))
                        nc.gpsimd.collective_compute(
                            "AllGather", mybir.AluOpType.bypass,
                            replica_groups=[list(range(NCORES))],
                            ins=[ag_in[g * BPC * 128:(g + 1) * BPC * 128, :]],
                            outs=[ag_out[g * NCORES * BPC * 128:(g + 1) * NCORES * BPC * 128, :]])

                # x_new external output
                nc.sync.dma_start(
                    out=xnew_out[:, :].rearrange("(b j) f -> j b f", j=128),
                    in_=xnew_sb[:, :].rearrange("j (b f) -> j b f", f=128))

            # =========================== stage 2 ===========================
            src2 = ag_out if with_collective else x_pi[:, :]
            with (
                tc.tile_pool(name="s2", bufs=3) as sp,
                tc.tile_pool(name="s2ea", bufs=2) as ep,
                tc.tile_pool(name="s2o", bufs=2) as op_,
                tc.tile_pool(name="s2g", bufs=4) as gp,
                tc.tile_pool(name="ps2_t", bufs=2, space="PSUM") as ps_t,
                tc.tile_pool(name="ps2_z", bufs=2, space="PSUM") as ps_z,
                (tc.For_i(0, reps, 1) if reps > 1 else _nullctx()),
            ):
                for b in range(n_blocks):
                    ea_blk = ep.tile([128, C], F32, tag="ea")
                    nc.sync.dma_start(out=ea_blk[:], in_=eaT[:, b * C:(b + 1) * C])
                    out_blk = op_.tile([128, C], F32, tag="ob")
                    for tt in range(TB):
                        t = b * TB + tt
                        cat = sp.tile([128, 2 * D], F32, tag="cat")
                        nc.gpsimd.indirect_dma_start(
                            out=cat[:, 0:128], out_offset=None, in_=src2,
                            in_offset=bass.IndirectOffsetOnAxis(ap=c_src[:, t:t + 1], axis=0))
                        # x_new[dst] = P^T selection from resident x_new block
                        pt = sp.tile([128, 128], F32, tag="pt")
                        nc.vector.tensor_tensor(
                            out=pt[:], in0=c_dstl[:, t:t + 1].to_broadcast([128, 128]),
                            in1=c_iota[:], op=mybir.AluOpType.is_equal)
                        p_ps = ps_t.tile([128, 128], F32, space="PSUM", tag="tp")
                        nc.tensor.transpose(out=p_ps[:], in_=pt[:], identity=c_id[:])
                        pm = sp.tile([128, 128], F32, tag="pm")
                        nc.scalar.copy(out=pm[:], in_=p_ps[:])
                        xd_ps = ps_t.tile([128, 128], F32, space="PSUM", tag="tp")
                        nc.tensor.matmul(out=xd_ps[:], lhsT=pm[:],
                                         rhs=xnew_sb[:, b * 128:(b + 1) * 128],
                                         start=True, stop=True)
                        nc.scalar.copy(out=cat[:, 128:256], in_=xd_ps[:])

                        # layernorm stats over 256
                        st6 = sp.tile([128, 6], F32, tag="st6")
                        nc.vector.bn_stats(out=st6[:], in_=cat[:])
                        mv = sp.tile([128, 2], F32, tag="mv")
                        nc.vector.bn_aggr(out=mv[:], in_=st6[:])
                        rstd = sp.tile([128, 1], F32, tag="rstd")
                        nc.scalar.activation(out=rstd[:], in_=mv[:, 1:2],
                                             func=mybir.ActivationFunctionType.Sqrt,
                                             bias=c_eps[:])
                        nc.vector.reciprocal(out=rstd[:], in_=rstd[:])
                        z = sp.tile([128, 2 * D], F32, tag="z")
                        nc.vector.tensor_scalar(
                            out=z[:], in0=cat[:], scalar1=mv[:, 0:1], scalar2=rstd[:],
                            op0=mybir.AluOpType.subtract, op1=mybir.AluOpType.mult)
                        if not fold_ln:
                            nc.vector.tensor_mul(out=z[:], in0=z[:], in1=c_lng[:])
                            nc.vector.tensor_add(out=z[:], in0=z[:], in1=c_lnb[:])
                        h = sp.tile([128, 2 * D], F32, tag="h")
                        nc.scalar.activation(out=h[:], in_=z[:],
                                             func=mybir.ActivationFunctionType.Relu)
                        # hT halves
                        h1_ps = ps_t.tile([128, 128], F32, space="PSUM", tag="tp")
                        nc.tensor.transpose(out=h1_ps[:], in_=h[:, 0:128], identity=c_id[:])
                        h1 = sp.tile([128, 128], F32, tag="h1")
                        nc.scalar.copy(out=h1[:], in_=h1_ps[:])
                        h2_ps = ps_t.tile([128, 128], F32, space="PSUM", tag="tp")
                        nc.tensor.transpose(out=h2_ps[:], in_=h[:, 128:256], identity=c_id[:])
                        h2 = sp.tile([128, 128], F32, tag="h2")
                        nc.vector.tensor_copy(out=h2[:], in_=h2_ps[:])
                        # z2T = Wm^T-chunks vs hT  -> [g, e]
                        z2_ps = ps_z.tile([128, 128], F32, space="PSUM", tag="z2")
                        nc.tensor.matmul(out=z2_ps[:], lhsT=c_wm[:, 0:128], rhs=h1[:],
                                         start=True, stop=False)
                        nc.tensor.matmul(out=z2_ps[:], lhsT=c_wm[:, 128:256], rhs=h2[:],
                                         start=False, stop=not has_bmlp)
                        if has_bmlp:
                            nc.tensor.matmul(out=z2_ps[:], lhsT=c_bmlp[:], rhs=c_ones[:],
                                             start=False, stop=True)
                        # residual in transposed layout
                        nc.vector.tensor_add(out=out_blk[:, tt * 128:(tt + 1) * 128],
                                             in0=z2_ps[:], in1=ea_blk[:, tt * 128:(tt + 1) * 128])
                    nc.sync.dma_start(out=eaT_out[:, b * C:(b + 1) * C], in_=out_blk[:])

    nc.compile()
    return nc


_PROGRAM_CACHE = {}


def _get_program(key):
    if key not in _PROGRAM_CACHE:
        _PROGRAM_CACHE[key] = build_program(*key)
    return _PROGRAM_CACHE[key]


# ------------------------------------------------------------------- kernel

def kernel(x, edge_index, edge_attr, W_l, b_l, W_r, b_r, W_e, b_e, att,
           bias, ln_g, ln_b, W_mlp, b_mlp):
    x = np.asarray(x, np.float32)
    ea = np.asarray(edge_attr, np.float32)
    prep = _prep(x, edge_index, ea)
    C, EP, T = prep["C"], prep["EP"], prep["T"]

    bsum = (np.asarray(b_l) + np.asarray(b_r) + np.asarray(b_e)).astype(np.float32)
    has_bsum = bool(np.any(bsum != 0))
    has_bias = bool(np.any(np.asarray(bias) != 0))
    ln_g = np.asarray(ln_g, np.float32)
    ln_b = np.asarray(ln_b, np.float32)
    fold_ln = bool(np.all(ln_g > 0))
    if fold_ln:
        wm = (ln_g[:, None] * np.asarray(W_mlp)).astype(np.float32)
        bb = ln_b / ln_g
        # relu(z*g+b) @ W = relu(z + b/g) @ (g*W) when g>0; absorb b/g:
        # relu(z + bb) -- only exact to absorb when bb == 0; else general path
        if np.any(bb != 0):
            fold_ln = False
    if not fold_ln:
        wm = np.asarray(W_mlp, np.float32)
    has_bmlp = bool(np.any(np.asarray(b_mlp) != 0))

    nc = _get_program((C, has_bsum, has_bias, has_bmlp, fold_ln))

    att_flat = np.asarray(att, np.float32).reshape(1, D)
    common = dict(
        x_pi=prep["x_pi"],
        w_l=np.asarray(W_l, np.float32), w_r=np.asarray(W_r, np.float32),
        w_e=np.asarray(W_e, np.float32), w_mlp=wm,
        att_b=np.tile(att_flat, (128, 1)),
        iota_f=np.tile(np.arange(128, dtype=np.float32)[None, :], (128, 1)),
        ident=np.eye(128, dtype=np.float32),
        rows1=np.ones((1, 128), np.float32),
        bsum_r=bsum.reshape(1, D),
        bias_b=np.tile(np.asarray(bias, np.float32).reshape(1, D), (128, 1)),
        bmlp_r=np.asarray(b_mlp, np.float32).reshape(1, D),
        lng_b=np.tile(ln_g.reshape(1, 2 * D), (128, 1)),
        lnb_b=np.tile(ln_b.reshape(1, 2 * D), (128, 1)),
    )
    in_maps = []
    for c in range(NCORES):
        m = dict(common)
        m["xT_loc"] = prep["xT_loc"][c]
        m["eaT"] = prep["eaT_dev"][c]
        m["xsT_all"] = prep["xsT_dev"][c]
        m["src_i"] = prep["src_dev"][c]
        m["dstloc"] = prep["dstloc_dev"][c]
        m["pen"] = prep["pen_dev"][c]
        in_maps.append(m)

    res = run_bass_kernel_spmd(nc, in_maps, core_ids=list(range(NCORES)))

    # unshard
    xnew = np.concatenate([res.results[c]["xnew_out"] for c in range(NCORES)],
                          axis=0)[:N]
    ea_new = np.empty((E, D), np.float32)
    pos = prep["pos_of_edge"]
    allout = np.stack([res.results[c]["eaT_out"] for c in range(NCORES)])  # [NC, D, EP]
    core = pos // EP
    col = pos % EP
    ea_new[:, :] = allout[core, :, col]
    return xnew, ea_new
